# revision 5
# baseline (speedup 1.0000x reference)
"""PointGroup kernel for 8 trn2 NeuronCores (self-contained).

Reference semantics reproduced bit-exactly:
  1) cluster_feats = segment_max(feats[point_ids], cluster_ids, 4096), with
     empty clusters -> 0.  cluster_ids sorted -> host builds a padded index
     matrix pidx[4096, CAP] (pad = first pid of each cluster so the max is
     unchanged); device gathers rows and tree-max reduces.  Exact.
  2) adjacency/connect per batch (B=4, P=2048).  All decisions are made in
     pre-sqrt (d2) space with rounding sequences matched op-for-op to what
     XLA-CPU emits for the reference (verified bit-exact on host):
       dot_rr : t0=fl(x_i*x_j); t1=fl(fl(y_i*y_j)+t0); t2=fl(fl(z_i*z_j)+t1)
       g      : fl(2*t2 - fl(n_i+n_j))        (negated d2; DVE round-round)
       knn    : g >= 8th-largest in row       (DVE max8 == top_k by value)
       valid  : g >= -A',  A' = largest f32 with fl(sqrt(A')) < 0.3f
       squeeze: g_shift > g_center            (pre-sqrt, validated)
       connect: (sem_i==sem_j) & (sem_i>=2)   (+ eye added on host)
     adjacency = (knn_i|knn_j) & valid & squeeze & connect; the diagonal is
     pure rounding noise (reference uses CPU FMA) and is computed by an
     exact FMA-emulation (Dekker/TwoSum) on per-point columns, output as a
     vector which the host scatters onto the diagonal.
Sharding: device d -> batch d//2, row half d%2 (the device works in a
permuted point order with its 1024 rows first; host un-permutes columns),
plus cluster range [d*512,(d+1)*512) for part 1.
"""
import numpy as np
from contextlib import ExitStack

import concourse.bass as bass
import concourse.bacc as bacc
import concourse.mybir as mybir
from concourse.tile import TileContext

F32 = mybir.dt.float32
U8 = mybir.dt.uint8
I32 = mybir.dt.int32
AOT = mybir.AluOpType
AFT = mybir.ActivationFunctionType

N, C = 262144, 32
NCLUST = 4096
B, P = 4, 2048
NBLK = P // 128          # 16 row blocks per batch
MYBLK = 8                # row blocks owned per device
HALFP = MYBLK * 128      # 1024
CLPD = NCLUST // 8       # clusters per device (512)
NCB = CLPD // 128        # cluster blocks per device (4)
CHUNK = 512              # phase-C column chunk
GSPLIT = 4               # gather sub-passes per cluster block
DEF_CAP = 192


def _a_prime():
    thr = np.float32(0.3)
    x = np.float32(thr * thr)
    while not np.sqrt(x) < thr:
        x = np.nextafter(x, np.float32(0.0), dtype=np.float32)
    while np.sqrt(np.nextafter(x, np.float32(1.0), dtype=np.float32)) < thr:
        x = np.nextafter(x, np.float32(1.0), dtype=np.float32)
    return x


def _emit_fma(nc, wp, y, t, tag):
    """fl(y*y + t) with a single rounding, via exact DVE ops (Dekker/TwoSum).
    y, t: [128, W] SBUF APs.  Returns the result tile."""
    shape = [y.shape[0], y.shape[1]]
    tt, ts = nc.vector.tensor_tensor, nc.vector.tensor_scalar

    def tl(sub):
        return wp.tile(shape, F32, tag=f"fma_{sub}", name=f"fma_{sub}")

    h = tl("h"); tt(out=h[:], in0=y, in1=y, op=AOT.mult)
    c = tl("c"); ts(out=c[:], in0=y, scalar1=4097.0, scalar2=None, op0=AOT.mult)
    d = tl("d"); tt(out=d[:], in0=c[:], in1=y, op=AOT.subtract)
    yh = tl("yh"); tt(out=yh[:], in0=c[:], in1=d[:], op=AOT.subtract)
    yl = tl("yl"); tt(out=yl[:], in0=y, in1=yh[:], op=AOT.subtract)
    p1 = tl("p1"); tt(out=p1[:], in0=yh[:], in1=yh[:], op=AOT.mult)
    e1 = tl("e1"); tt(out=e1[:], in0=p1[:], in1=h[:], op=AOT.subtract)
    p2 = tl("p2"); tt(out=p2[:], in0=yh[:], in1=yl[:], op=AOT.mult)
    p22 = tl("p22"); ts(out=p22[:], in0=p2[:], scalar1=2.0, scalar2=None, op0=AOT.mult)
    e2 = tl("e2"); tt(out=e2[:], in0=e1[:], in1=p22[:], op=AOT.add)
    p3 = tl("p3"); tt(out=p3[:], in0=yl[:], in1=yl[:], op=AOT.mult)
    e = tl("e"); tt(out=e[:], in0=e2[:], in1=p3[:], op=AOT.add)
    s = tl("s"); tt(out=s[:], in0=h[:], in1=t, op=AOT.add)
    z = tl("z"); tt(out=z[:], in0=s[:], in1=h[:], op=AOT.subtract)
    w1 = tl("w1"); tt(out=w1[:], in0=s[:], in1=z[:], op=AOT.subtract)
    w2 = tl("w2"); tt(out=w2[:], in0=h[:], in1=w1[:], op=AOT.subtract)
    w3 = tl("w3"); tt(out=w3[:], in0=t, in1=z[:], op=AOT.subtract)
    er = tl("er"); tt(out=er[:], in0=w2[:], in1=w3[:], op=AOT.add)
    ee = tl("ee"); tt(out=ee[:], in0=er[:], in1=e[:], op=AOT.add)
    r = tl("r"); tt(out=r[:], in0=s[:], in1=ee[:], op=AOT.add)
    return r


def build_program(cap):
    nc = bacc.Bacc("TRN2", target_bir_lowering=False)
    A = float(_a_prime())

    din = {}
    def dram_in(name, shape, dt=F32):
        din[name] = nc.dram_tensor(name, list(shape), dt, kind="ExternalInput")

    dram_in("feats", (N, C))
    dram_in("pidx", (128, NCB * cap), I32)
    dram_in("emask", (128, NCB))
    for pre in ("s", "c"):
        for ax in ("x", "y", "z", "n"):
            dram_in(f"{pre}{ax}_r", (P,))
            dram_in(f"{pre}{ax}_c", (128, NBLK))
    dram_in("sem_r", (P,))
    dram_in("sem_c", (128, NBLK))
    dram_in("e2_c", (128, NBLK))

    do = {
        "cfeat": nc.dram_tensor("cfeat", [CLPD, C], F32, kind="ExternalOutput"),
        "adj": nc.dram_tensor("adj", [HALFP, P], F32, kind="ExternalOutput"),
        "conn": nc.dram_tensor("conn", [HALFP, P], U8, kind="ExternalOutput"),
        "diag": nc.dram_tensor("diag", [128, NBLK], F32, kind="ExternalOutput"),
    }

    with TileContext(nc) as tc, ExitStack() as ctx:
        cp = ctx.enter_context(tc.tile_pool(name="cp", bufs=1))   # persistent
        wp = ctx.enter_context(tc.tile_pool(name="wp", bufs=2))   # working
        kp = ctx.enter_context(tc.tile_pool(name="kp", bufs=1))   # gsh keep
        gp = ctx.enter_context(tc.tile_pool(name="gp", bufs=2))   # gather
        dr = ctx.enter_context(tc.tile_pool(name="dr", bufs=1, space="DRAM"))

        def bcast(name, tag):
            t = cp.tile([128, P], F32, tag=tag, name=tag)
            nc.sync.dma_start(out=t[:],
                              in_=din[name].ap()[None, :].to_broadcast([128, P]))
            return t

        def loadt(name, dt=F32, w=NBLK):
            t = cp.tile([128, w], dt, tag=name, name=name)
            nc.sync.dma_start(out=t[:], in_=din[name].ap())
            return t

        XJ = bcast("sx_r", "bx"); YJ = bcast("sy_r", "by")
        ZJ = bcast("sz_r", "bz"); NJ = bcast("sn_r", "bn")
        SEMJ = bcast("sem_r", "bsem")
        sx_c = loadt("sx_c"); sy_c = loadt("sy_c")
        sz_c = loadt("sz_c"); sn_c = loadt("sn_c")
        cx_c = loadt("cx_c"); cy_c = loadt("cy_c")
        cz_c = loadt("cz_c"); cn_c = loadt("cn_c")
        sem_c = loadt("sem_c"); e2_c = loadt("e2_c")
        gk8_dram = dr.tile([P], F32)

        def chain(xj, yj, zj, nj, xc, yc, zc, n_c, blk, out_ap, cols=slice(0, P)):
            """out = fl(2*dot_rr - fl(n_i+n_j)) for row-block blk, cols."""
            w = cols.stop - cols.start
            t0 = wp.tile([128, P], F32, tag="t0")
            nc.scalar.activation(out=t0[:, :w], in_=xj[:, cols], func=AFT.Copy,
                                 bias=0.0, scale=xc[:, blk:blk + 1])
            t1 = wp.tile([128, P], F32, tag="t1")
            nc.vector.scalar_tensor_tensor(out=t1[:, :w], in0=yj[:, cols],
                                           scalar=yc[:, blk:blk + 1], in1=t0[:, :w],
                                           op0=AOT.mult, op1=AOT.add)
            t2 = wp.tile([128, P], F32, tag="t0")
            nc.vector.scalar_tensor_tensor(out=t2[:, :w], in0=zj[:, cols],
                                           scalar=zc[:, blk:blk + 1], in1=t1[:, :w],
                                           op0=AOT.mult, op1=AOT.add)
            S = wp.tile([128, P], F32, tag="t1")
            nc.scalar.activation(out=S[:, :w], in_=nj[:, cols], func=AFT.Relu,
                                 bias=n_c[:, blk:blk + 1], scale=1.0)
            nc.vector.scalar_tensor_tensor(out=out_ap, in0=t2[:, :w], scalar=2.0,
                                           in1=S[:, :w], op0=AOT.mult,
                                           op1=AOT.subtract)

        # ---------- phase A: g_shift + kth for all 16 row blocks ----------
        gsh_keep, gk8_cols = [], []
        for blk in range(NBLK):
            keep = blk < MYBLK
            g = (kp.tile([128, P], F32, tag=f"gk{blk}", name=f"gk{blk}") if keep
                 else wp.tile([128, P], F32, tag="gtmp", name="gtmp", bufs=1))
            chain(XJ, YJ, ZJ, NJ, sx_c, sy_c, sz_c, sn_c, blk, g[:])
            m8 = wp.tile([128, 8], F32, tag="m8")
            nc.vector.max(out=m8[:], in_=g[:])
            gcol = (kp.tile([128, 1], F32, tag=f"g8_{blk}", name=f"g8_{blk}") if keep
                    else wp.tile([128, 1], F32, tag="g8tmp", name="g8tmp"))
            nc.vector.tensor_scalar(out=gcol[:], in0=m8[:, 7:8], scalar1=-A,
                                    scalar2=None, op0=AOT.max)
            if keep:
                gsh_keep.append(g); gk8_cols.append(gcol)
            nc.sync.dma_start(out=gk8_dram[blk * 128:(blk + 1) * 128, None],
                              in_=gcol[:])

        GK8 = cp.tile([128, P], F32, tag="bgk8")
        nc.sync.dma_start(out=GK8[:],
                          in_=gk8_dram[None, :].to_broadcast([128, P]))

        # c-side broadcasts reuse the (now dead) shift slots
        XJc = bcast("cx_r", "bx"); YJc = bcast("cy_r", "by")
        ZJc = bcast("cz_r", "bz"); NJc = bcast("cn_r", "bn")

        # ---------- phase C: masks + outputs for my 8 row blocks ----------
        for blk in range(MYBLK):
            gsh, g8c = gsh_keep[blk], gk8_cols[blk]
            for ci in range(P // CHUNK):
                cols = slice(ci * CHUNK, (ci + 1) * CHUNK)
                gc = wp.tile([128, CHUNK], F32, tag="gc")
                chain(XJc, YJc, ZJc, NJc, cx_c, cy_c, cz_c, cn_c, blk,
                      gc[:], cols)
                kc = wp.tile([128, CHUNK], F32, tag="kc")
                nc.vector.tensor_tensor(out=kc[:], in0=GK8[:, cols],
                                        in1=g8c[:, :1].to_broadcast([128, CHUNK]),
                                        op=AOT.min)
                am = wp.tile([128, CHUNK], F32, tag="am")
                nc.vector.tensor_tensor(out=am[:], in0=gsh[:, cols], in1=kc[:],
                                        op=AOT.is_ge)
                cr = wp.tile([128, CHUNK], F32, tag="cr")
                nc.vector.tensor_tensor(out=cr[:], in0=gsh[:, cols], in1=gc[:],
                                        op=AOT.is_gt)
                e12 = wp.tile([128, CHUNK], F32, tag="e12")
                nc.vector.tensor_scalar(out=e12[:], in0=SEMJ[:, cols],
                                        scalar1=sem_c[:, blk:blk + 1],
                                        scalar2=e2_c[:, blk:blk + 1],
                                        op0=AOT.is_equal, op1=AOT.mult)
                ta = wp.tile([128, CHUNK], F32, tag="ta")
                nc.vector.tensor_tensor(out=ta[:], in0=am[:], in1=cr[:], op=AOT.add)
                adjf = wp.tile([128, CHUNK], F32, tag="adjf")
                nc.vector.scalar_tensor_tensor(out=adjf[:], in0=ta[:], scalar=1.5,
                                               in1=e12[:], op0=AOT.is_ge,
                                               op1=AOT.mult)
                nc.sync.dma_start(
                    out=do["adj"].ap()[blk * 128:(blk + 1) * 128, cols], in_=adjf[:])
                cu8 = wp.tile([128, CHUNK], U8, tag="cu8")
                nc.vector.tensor_copy(out=cu8[:], in_=e12[:])
                nc.sync.dma_start(
                    out=do["conn"].ap()[blk * 128:(blk + 1) * 128, cols], in_=cu8[:])

        # ---------- phase D: exact diagonal ----------
        def diag_m(xc, yc, zc, n_c, tag):
            t0 = wp.tile([128, NBLK], F32, tag="fma_t0")
            nc.vector.tensor_tensor(out=t0[:], in0=xc[:], in1=xc[:], op=AOT.mult)
            f1 = _emit_fma(nc, wp, yc[:], t0[:], tag)
            f2 = _emit_fma(nc, wp, zc[:], f1[:], tag)
            u1 = wp.tile([128, NBLK], F32, tag="fma_u1")
            nc.vector.tensor_scalar(out=u1[:], in0=n_c[:], scalar1=2.0,
                                    scalar2=None, op0=AOT.mult)
            u2 = wp.tile([128, NBLK], F32, tag="fma_u2")
            nc.vector.tensor_scalar(out=u2[:], in0=f2[:], scalar1=2.0,
                                    scalar2=None, op0=AOT.mult)
            d2 = wp.tile([128, NBLK], F32, tag="fma_d2")
            nc.vector.tensor_tensor(out=d2[:], in0=u1[:], in1=u2[:], op=AOT.subtract)
            m = wp.tile([128, NBLK], F32, tag=f"fma_m_{tag}")
            nc.vector.tensor_scalar(out=m[:], in0=d2[:], scalar1=0.0,
                                    scalar2=None, op0=AOT.max)
            return m

        m_sh = diag_m(sx_c, sy_c, sz_c, sn_c, "dsh")
        m_c = diag_m(cx_c, cy_c, cz_c, cn_c, "dc")
        dpred = wp.tile([128, NBLK], F32, tag="dpred")
        nc.vector.tensor_tensor(out=dpred[:], in0=m_sh[:], in1=m_c[:], op=AOT.is_lt)
        nc.sync.dma_start(out=do["diag"].ap(), in_=dpred[:])

        # ---------- phase E: segment max (padded gather + tree reduce) ----
        pidx_sb = loadt("pidx", dt=I32, w=NCB * cap)
        emask_sb = loadt("emask", w=NCB)
        sub = cap // GSPLIT
        for cb in range(NCB):
            parts = []
            for gs in range(GSPLIT):
                G = gp.tile([128, sub * C], F32, tag="G")
                for k in range(sub):
                    kk = cb * cap + gs * sub + k
                    nc.gpsimd.indirect_dma_start(
                        out=G[:, k * C:(k + 1) * C], out_offset=None,
                        in_=din["feats"].ap(),
                        in_offset=bass.IndirectOffsetOnAxis(
                            ap=pidx_sb[:, kk:kk + 1], axis=0))
                h = sub
                while h > 1:
                    if h % 2 == 0:
                        h //= 2
                        nc.vector.tensor_tensor(out=G[:, :h * C], in0=G[:, :h * C],
                                                in1=G[:, h * C:2 * h * C], op=AOT.max)
                    else:
                        nc.vector.tensor_tensor(out=G[:, :C], in0=G[:, :C],
                                                in1=G[:, (h - 1) * C:h * C],
                                                op=AOT.max)
                        h -= 1
                r = wp.tile([128, C], F32, tag=f"gr{gs}")
                nc.vector.tensor_copy(out=r[:], in_=G[:, :C])
                parts.append(r)
            out = wp.tile([128, C], F32, tag="cfo")
            nc.vector.tensor_tensor(out=out[:], in0=parts[0][:], in1=parts[1][:],
                                    op=AOT.max)
            for r_extra in parts[2:]:
                nc.vector.tensor_tensor(out=out[:], in0=out[:], in1=r_extra[:],
                                        op=AOT.max)
            nc.vector.tensor_scalar(out=out[:], in0=out[:],
                                    scalar1=emask_sb[:, cb:cb + 1],
                                    scalar2=None, op0=AOT.mult)
            nc.sync.dma_start(out=do["cfeat"].ap()[cb * 128:(cb + 1) * 128, :],
                              in_=out[:])

    nc.compile()
    return nc


_prog_cache = {}


def _get_prog(cap):
    if cap not in _prog_cache:
        _prog_cache[cap] = build_program(cap)
    return _prog_cache[cap]


def _prep_inputs(feats, cluster_ids, point_ids, centers, offsets, sem, cap):
    counts = np.bincount(cluster_ids.astype(np.int64), minlength=NCLUST).astype(np.int64)
    starts = np.zeros(NCLUST, dtype=np.int64)
    np.cumsum(counts[:-1], out=starts[1:])
    emask = (counts > 0).astype(np.float32)
    k = np.arange(cap)[None, :]
    gidx = starts[:, None] + np.minimum(k, np.maximum(counts, 1)[:, None] - 1)
    pidx = point_ids[gidx].astype(np.int32)

    shifts = (centers + offsets).astype(np.float32)
    semf = sem.astype(np.float32)

    def norms(a):
        f = np.float32
        return ((a[:, 0] * a[:, 0]).astype(f) + (a[:, 1] * a[:, 1]).astype(f)
                ).astype(f) + (a[:, 2] * a[:, 2]).astype(f)

    def cols(arr):
        return np.ascontiguousarray(arr.reshape(NBLK, 128).T)

    in_maps = []
    for d in range(8):
        b, half = d // 2, d % 2
        # permuted point order: my 1024 rows first
        q = np.r_[np.arange(half * HALFP, half * HALFP + HALFP),
                  np.arange((1 - half) * HALFP, (1 - half) * HALFP + HALFP)]
        cv = centers.reshape(B, P, 3)[b][q]
        sv = shifts.reshape(B, P, 3)[b][q]
        smv = semf.reshape(B, P)[b][q]
        m = {"feats": feats}
        cl0 = d * CLPD
        m["pidx"] = np.ascontiguousarray(
            pidx[cl0:cl0 + CLPD].reshape(NCB, 128, cap).transpose(1, 0, 2)
            .reshape(128, NCB * cap))
        m["emask"] = np.ascontiguousarray(emask[cl0:cl0 + CLPD].reshape(NCB, 128).T)
        for pre, view in (("s", sv), ("c", cv)):
            nv = norms(view).astype(np.float32)
            for ax, arr in (("x", view[:, 0]), ("y", view[:, 1]),
                            ("z", view[:, 2]), ("n", nv)):
                arr = np.ascontiguousarray(np.asarray(arr, dtype=np.float32))
                m[f"{pre}{ax}_r"] = arr
                m[f"{pre}{ax}_c"] = cols(arr)
        m["sem_r"] = np.ascontiguousarray(smv)
        m["sem_c"] = cols(smv)
        m["e2_c"] = cols((smv >= 2.0).astype(np.float32))
        in_maps.append(m)
    return in_maps


def kernel(feats, cluster_ids, point_ids, overseg_centers, overseg_offsets,
           overseg_semantic, _trace=False):
    from concourse.bass_utils import run_bass_kernel_spmd

    feats = np.ascontiguousarray(np.asarray(feats, dtype=np.float32))
    cluster_ids = np.asarray(cluster_ids).astype(np.int64)
    point_ids = np.asarray(point_ids).astype(np.int64)
    centers = np.asarray(overseg_centers, dtype=np.float32)
    offsets = np.asarray(overseg_offsets, dtype=np.float32)
    sem = np.asarray(overseg_semantic).astype(np.int64)

    counts = np.bincount(cluster_ids, minlength=NCLUST)
    cap = DEF_CAP if counts.max() <= DEF_CAP else int(32 * np.ceil(counts.max() / 32))

    nc = _get_prog(cap)
    in_maps = _prep_inputs(feats, cluster_ids, point_ids, centers, offsets, sem, cap)
    try:
        res = run_bass_kernel_spmd(nc, in_maps, core_ids=list(range(8)),
                                   trace=_trace)
    except ModuleNotFoundError:
        res = run_bass_kernel_spmd(nc, in_maps, core_ids=list(range(8)),
                                   trace=False)
    outs = res.results

    cluster_feats = np.concatenate([outs[d]["cfeat"] for d in range(8)], axis=0)
    adjacency = np.empty((B, P, P), dtype=np.float32)
    connect = np.empty((B, P, P), dtype=bool)
    for d in range(8):
        b, half = d // 2, d % 2
        r0 = half * HALFP
        adj = outs[d]["adj"]
        conn = outs[d]["conn"].astype(bool)
        if half == 1:   # device columns are in permuted order: un-swap halves
            adj = np.concatenate([adj[:, HALFP:], adj[:, :HALFP]], axis=1)
            conn = np.concatenate([conn[:, HALFP:], conn[:, :HALFP]], axis=1)
        adjacency[b, r0:r0 + HALFP, :] = adj
        connect[b, r0:r0 + HALFP, :] = conn
        dv = outs[d]["diag"][:, :MYBLK]       # my blocks are cols 0..7
        vec = np.ascontiguousarray(dv.T).reshape(HALFP)
        idx = np.arange(r0, r0 + HALFP)
        adjacency[b, idx, idx] = vec
        connect[b, idx, idx] = True
    if _trace:
        kernel._last_results = res
    return cluster_feats, adjacency, connect


# revision 9
# speedup vs baseline: 245242.0183x; 245242.0183x over previous
"""PointGroup kernel for 8 trn2 NeuronCores (self-contained).

Reference semantics reproduced bit-exactly:
  1) cluster_feats = segment_max(feats[point_ids], cluster_ids, 4096), with
     empty clusters -> 0.  cluster_ids sorted -> host builds a padded index
     matrix pidx[4096, CAP] (pad = first pid of each cluster so the max is
     unchanged); device gathers rows and tree-max reduces.  Exact.
  2) adjacency/connect per batch (B=4, P=2048).  All decisions are made in
     pre-sqrt (d2) space with rounding sequences matched op-for-op to what
     XLA-CPU emits for the reference (verified bit-exact on host):
       dot_rr : t0=fl(x_i*x_j); t1=fl(fl(y_i*y_j)+t0); t2=fl(fl(z_i*z_j)+t1)
       g      : fl(2*t2 - fl(n_i+n_j))        (negated d2; DVE round-round)
       knn    : g >= 8th-largest in row       (DVE max8 == top_k by value)
       valid  : g >= -A',  A' = largest f32 with fl(sqrt(A')) < 0.3f
       squeeze: g_shift > g_center            (pre-sqrt, validated)
       connect: (sem_i==sem_j) & (sem_i>=2)   (+ eye added on host)
     adjacency = (knn_i|knn_j) & valid & squeeze & connect; the diagonal is
     pure rounding noise (reference uses CPU FMA) and is computed by an
     exact FMA-emulation (Dekker/TwoSum) on per-point columns, output as a
     vector which the host scatters onto the diagonal.
Sharding: device d -> batch d//2, row half d%2 (the device works in a
permuted point order with its 1024 rows first; host un-permutes columns),
plus cluster range [d*512,(d+1)*512) for part 1.
"""
import numpy as np
from contextlib import ExitStack

import concourse.bass as bass
import concourse.bacc as bacc
import concourse.mybir as mybir
from concourse.tile import TileContext

F32 = mybir.dt.float32
U8 = mybir.dt.uint8
I32 = mybir.dt.int32
AOT = mybir.AluOpType
AFT = mybir.ActivationFunctionType

N, C = 262144, 32
NCLUST = 4096
B, P = 4, 2048
NBLK = P // 128          # 16 row blocks per batch
MYBLK = 8                # row blocks owned per device
HALFP = MYBLK * 128      # 1024
CLPD = NCLUST // 8       # clusters per device (512)
NCB = CLPD // 128        # cluster blocks per device (4)
CHUNK = 512              # phase-C column chunk
GSPLIT = 2               # gather sub-passes per cluster block
DEF_CAP = 192


def _a_prime():
    thr = np.float32(0.3)
    x = np.float32(thr * thr)
    while not np.sqrt(x) < thr:
        x = np.nextafter(x, np.float32(0.0), dtype=np.float32)
    while np.sqrt(np.nextafter(x, np.float32(1.0), dtype=np.float32)) < thr:
        x = np.nextafter(x, np.float32(1.0), dtype=np.float32)
    return x


def _emit_fma(nc, wp, y, t, tag):
    """fl(y*y + t) with a single rounding, via exact DVE ops (Dekker/TwoSum).
    y, t: [128, W] SBUF APs.  Returns the result tile."""
    shape = [y.shape[0], y.shape[1]]
    tt, ts = nc.vector.tensor_tensor, nc.vector.tensor_scalar

    def tl(sub):
        return wp.tile(shape, F32, tag=f"fma_{sub}", name=f"fma_{sub}")

    h = tl("h"); tt(out=h[:], in0=y, in1=y, op=AOT.mult)
    c = tl("c"); ts(out=c[:], in0=y, scalar1=4097.0, scalar2=None, op0=AOT.mult)
    d = tl("d"); tt(out=d[:], in0=c[:], in1=y, op=AOT.subtract)
    yh = tl("yh"); tt(out=yh[:], in0=c[:], in1=d[:], op=AOT.subtract)
    yl = tl("yl"); tt(out=yl[:], in0=y, in1=yh[:], op=AOT.subtract)
    p1 = tl("p1"); tt(out=p1[:], in0=yh[:], in1=yh[:], op=AOT.mult)
    e1 = tl("e1"); tt(out=e1[:], in0=p1[:], in1=h[:], op=AOT.subtract)
    p2 = tl("p2"); tt(out=p2[:], in0=yh[:], in1=yl[:], op=AOT.mult)
    p22 = tl("p22"); ts(out=p22[:], in0=p2[:], scalar1=2.0, scalar2=None, op0=AOT.mult)
    e2 = tl("e2"); tt(out=e2[:], in0=e1[:], in1=p22[:], op=AOT.add)
    p3 = tl("p3"); tt(out=p3[:], in0=yl[:], in1=yl[:], op=AOT.mult)
    e = tl("e"); tt(out=e[:], in0=e2[:], in1=p3[:], op=AOT.add)
    s = tl("s"); tt(out=s[:], in0=h[:], in1=t, op=AOT.add)
    z = tl("z"); tt(out=z[:], in0=s[:], in1=h[:], op=AOT.subtract)
    w1 = tl("w1"); tt(out=w1[:], in0=s[:], in1=z[:], op=AOT.subtract)
    w2 = tl("w2"); tt(out=w2[:], in0=h[:], in1=w1[:], op=AOT.subtract)
    w3 = tl("w3"); tt(out=w3[:], in0=t, in1=z[:], op=AOT.subtract)
    er = tl("er"); tt(out=er[:], in0=w2[:], in1=w3[:], op=AOT.add)
    ee = tl("ee"); tt(out=ee[:], in0=er[:], in1=e[:], op=AOT.add)
    r = tl("r"); tt(out=r[:], in0=s[:], in1=ee[:], op=AOT.add)
    return r


def build_program(cap):
    nc = bacc.Bacc("TRN2", target_bir_lowering=False)
    A = float(_a_prime())

    din = {}
    def dram_in(name, shape, dt=F32):
        din[name] = nc.dram_tensor(name, list(shape), dt, kind="ExternalInput")

    dram_in("feats", (N, C))
    dram_in("pidx", (128, NCB * cap), I32)
    dram_in("emask", (128, NCB))
    for pre in ("s", "c"):
        for ax in ("x", "y", "z", "n"):
            dram_in(f"{pre}{ax}_r", (P,))
            dram_in(f"{pre}{ax}_c", (128, NBLK))
    dram_in("sem_r", (P,))
    dram_in("sem_c", (128, NBLK))
    dram_in("e2_c", (128, NBLK))
    for ax in ("x", "y", "z", "n"):
        dram_in(f"dg{ax}", (128, 2 * NBLK))

    do = {
        "cfeat": nc.dram_tensor("cfeat", [CLPD, C], F32, kind="ExternalOutput"),
        "adj": nc.dram_tensor("adj", [HALFP, P], F32, kind="ExternalOutput"),
        "conn": nc.dram_tensor("conn", [HALFP, P], U8, kind="ExternalOutput"),
        "diag": nc.dram_tensor("diag", [128, NBLK], F32, kind="ExternalOutput"),
    }

    with TileContext(nc) as tc, ExitStack() as ctx:
        cp = ctx.enter_context(tc.tile_pool(name="cp", bufs=1))   # persistent
        wp = ctx.enter_context(tc.tile_pool(name="wp", bufs=2))   # working
        kp = ctx.enter_context(tc.tile_pool(name="kp", bufs=1))   # gsh keep
        gp = ctx.enter_context(tc.tile_pool(name="gp", bufs=1))   # gather
        dr = ctx.enter_context(tc.tile_pool(name="dr", bufs=1, space="DRAM"))

        def bcast(name, tag):
            t = cp.tile([128, P], F32, tag=tag, name=tag)
            nc.sync.dma_start(out=t[:],
                              in_=din[name].ap()[None, :].to_broadcast([128, P]))
            return t

        def loadt(name, dt=F32, w=NBLK):
            t = cp.tile([128, w], dt, tag=name, name=name)
            nc.sync.dma_start(out=t[:], in_=din[name].ap())
            return t

        XJ = bcast("sx_r", "bx"); YJ = bcast("sy_r", "by")
        ZJ = bcast("sz_r", "bz"); NJ = bcast("sn_r", "bn")
        SEMJ = bcast("sem_r", "bsem")
        sx_c = loadt("sx_c"); sy_c = loadt("sy_c")
        sz_c = loadt("sz_c"); sn_c = loadt("sn_c")
        cx_c = loadt("cx_c"); cy_c = loadt("cy_c")
        cz_c = loadt("cz_c"); cn_c = loadt("cn_c")
        sem_c = loadt("sem_c"); e2_c = loadt("e2_c")
        gk8_dram = dr.tile([P], F32)

        def chain(xj, yj, zj, nj, xc, yc, zc, n_c, blk, out_ap, cols=slice(0, P)):
            """out = fl(2*dot_rr - fl(n_i+n_j)) for row-block blk, cols."""
            w = cols.stop - cols.start
            t0 = wp.tile([128, P], F32, tag="t0")
            nc.scalar.activation(out=t0[:, :w], in_=xj[:, cols], func=AFT.Copy,
                                 bias=0.0, scale=xc[:, blk:blk + 1])
            t1 = wp.tile([128, P], F32, tag="t1")
            nc.vector.scalar_tensor_tensor(out=t1[:, :w], in0=yj[:, cols],
                                           scalar=yc[:, blk:blk + 1], in1=t0[:, :w],
                                           op0=AOT.mult, op1=AOT.add)
            t2 = wp.tile([128, P], F32, tag="t0")
            nc.vector.scalar_tensor_tensor(out=t2[:, :w], in0=zj[:, cols],
                                           scalar=zc[:, blk:blk + 1], in1=t1[:, :w],
                                           op0=AOT.mult, op1=AOT.add)
            S = wp.tile([128, P], F32, tag="t1")
            nc.scalar.activation(out=S[:, :w], in_=nj[:, cols], func=AFT.Relu,
                                 bias=n_c[:, blk:blk + 1], scale=1.0)
            nc.vector.scalar_tensor_tensor(out=out_ap, in0=t2[:, :w], scalar=2.0,
                                           in1=S[:, :w], op0=AOT.mult,
                                           op1=AOT.subtract)

        # ---------- phase A: g_shift + kth for all 16 row blocks ----------
        gsh_keep, gk8_cols = [], []
        for blk in range(NBLK):
            keep = blk < MYBLK
            g = (kp.tile([128, P], F32, tag=f"gk{blk}", name=f"gk{blk}") if keep
                 else wp.tile([128, P], F32, tag="gtmp", name="gtmp", bufs=1))
            chain(XJ, YJ, ZJ, NJ, sx_c, sy_c, sz_c, sn_c, blk, g[:])
            m8 = wp.tile([128, 8], F32, tag="m8")
            nc.vector.max(out=m8[:], in_=g[:])
            gcol = (kp.tile([128, 1], F32, tag=f"g8_{blk}", name=f"g8_{blk}") if keep
                    else wp.tile([128, 1], F32, tag="g8tmp", name="g8tmp"))
            nc.vector.tensor_scalar(out=gcol[:], in0=m8[:, 7:8], scalar1=-A,
                                    scalar2=None, op0=AOT.max)
            if keep:
                gsh_keep.append(g); gk8_cols.append(gcol)
            nc.sync.dma_start(out=gk8_dram[blk * 128:(blk + 1) * 128, None],
                              in_=gcol[:])

        GK8 = cp.tile([128, P], F32, tag="bgk8")
        nc.sync.dma_start(out=GK8[:],
                          in_=gk8_dram[None, :].to_broadcast([128, P]))

        # c-side broadcasts reuse the (now dead) shift slots
        XJc = bcast("cx_r", "bx"); YJc = bcast("cy_r", "by")
        ZJc = bcast("cz_r", "bz"); NJc = bcast("cn_r", "bn")

        # ---------- phase C: masks + outputs for my 8 row blocks ----------
        for blk in range(MYBLK):
            gsh, g8c = gsh_keep[blk], gk8_cols[blk]
            for ci in range(P // CHUNK):
                cols = slice(ci * CHUNK, (ci + 1) * CHUNK)
                gc = wp.tile([128, CHUNK], F32, tag="gc")
                chain(XJc, YJc, ZJc, NJc, cx_c, cy_c, cz_c, cn_c, blk,
                      gc[:], cols)
                kc = wp.tile([128, CHUNK], F32, tag="kc")
                nc.vector.tensor_tensor(out=kc[:], in0=GK8[:, cols],
                                        in1=g8c[:, :1].to_broadcast([128, CHUNK]),
                                        op=AOT.min)
                am = wp.tile([128, CHUNK], F32, tag="am")
                nc.vector.tensor_tensor(out=am[:], in0=gsh[:, cols], in1=kc[:],
                                        op=AOT.is_ge)
                cr = wp.tile([128, CHUNK], F32, tag="cr")
                nc.vector.tensor_tensor(out=cr[:], in0=gsh[:, cols], in1=gc[:],
                                        op=AOT.is_gt)
                e12 = wp.tile([128, CHUNK], F32, tag="e12")
                nc.vector.tensor_scalar(out=e12[:], in0=SEMJ[:, cols],
                                        scalar1=sem_c[:, blk:blk + 1],
                                        scalar2=e2_c[:, blk:blk + 1],
                                        op0=AOT.is_equal, op1=AOT.mult)
                ta = wp.tile([128, CHUNK], F32, tag="ta")
                nc.vector.tensor_tensor(out=ta[:], in0=am[:], in1=cr[:], op=AOT.add)
                adjf = wp.tile([128, CHUNK], F32, tag="adjf")
                nc.vector.scalar_tensor_tensor(out=adjf[:], in0=ta[:], scalar=1.5,
                                               in1=e12[:], op0=AOT.is_ge,
                                               op1=AOT.mult)
                nc.sync.dma_start(
                    out=do["adj"].ap()[blk * 128:(blk + 1) * 128, cols], in_=adjf[:])
                cu8 = wp.tile([128, CHUNK], U8, tag="cu8")
                nc.scalar.copy(out=cu8[:], in_=e12[:])
                nc.sync.dma_start(
                    out=do["conn"].ap()[blk * 128:(blk + 1) * 128, cols], in_=cu8[:])

        # ---------- phase D: exact diagonal ----------
        dgx = loadt("dgx", w=2 * NBLK); dgy = loadt("dgy", w=2 * NBLK)
        dgz = loadt("dgz", w=2 * NBLK); dgn = loadt("dgn", w=2 * NBLK)
        t0 = wp.tile([128, 2 * NBLK], F32, tag="fma_t0")
        nc.vector.tensor_tensor(out=t0[:], in0=dgx[:], in1=dgx[:], op=AOT.mult)
        f1 = _emit_fma(nc, wp, dgy[:], t0[:], "dg1")
        f2 = _emit_fma(nc, wp, dgz[:], f1[:], "dg2")
        u1 = wp.tile([128, 2 * NBLK], F32, tag="fma_u1")
        nc.vector.tensor_scalar(out=u1[:], in0=dgn[:], scalar1=2.0,
                                scalar2=None, op0=AOT.mult)
        u2 = wp.tile([128, 2 * NBLK], F32, tag="fma_u2")
        nc.vector.tensor_scalar(out=u2[:], in0=f2[:], scalar1=2.0,
                                scalar2=None, op0=AOT.mult)
        d2 = wp.tile([128, 2 * NBLK], F32, tag="fma_d2")
        nc.vector.tensor_tensor(out=d2[:], in0=u1[:], in1=u2[:], op=AOT.subtract)
        mm = wp.tile([128, 2 * NBLK], F32, tag="fma_m")
        nc.vector.tensor_scalar(out=mm[:], in0=d2[:], scalar1=0.0,
                                scalar2=None, op0=AOT.max)
        dpred = wp.tile([128, NBLK], F32, tag="dpred")
        nc.vector.tensor_tensor(out=dpred[:], in0=mm[:, :NBLK],
                                in1=mm[:, NBLK:], op=AOT.is_lt)
        nc.sync.dma_start(out=do["diag"].ap(), in_=dpred[:])

        # ---------- phase E: segment max (padded gather + tree reduce) ----
        pidx_sb = loadt("pidx", dt=I32, w=NCB * cap)
        emask_sb = loadt("emask", w=NCB)
        sub = cap // GSPLIT
        for cb in range(NCB):
            parts = []
            for gs in range(GSPLIT):
                G = gp.tile([128, sub * C], F32, tag="G")
                for k in range(sub):
                    kk = cb * cap + gs * sub + k
                    nc.gpsimd.indirect_dma_start(
                        out=G[:, k * C:(k + 1) * C], out_offset=None,
                        in_=din["feats"].ap(),
                        in_offset=bass.IndirectOffsetOnAxis(
                            ap=pidx_sb[:, kk:kk + 1], axis=0))
                h = sub
                while h > 1:
                    if h % 2 == 0:
                        h //= 2
                        nc.vector.tensor_tensor(out=G[:, :h * C], in0=G[:, :h * C],
                                                in1=G[:, h * C:2 * h * C], op=AOT.max)
                    else:
                        nc.vector.tensor_tensor(out=G[:, :C], in0=G[:, :C],
                                                in1=G[:, (h - 1) * C:h * C],
                                                op=AOT.max)
                        h -= 1
                r = wp.tile([128, C], F32, tag=f"gr{gs}")
                nc.vector.tensor_copy(out=r[:], in_=G[:, :C])
                parts.append(r)
            out = wp.tile([128, C], F32, tag="cfo")
            nc.vector.tensor_tensor(out=out[:], in0=parts[0][:], in1=parts[1][:],
                                    op=AOT.max)
            for r_extra in parts[2:]:
                nc.vector.tensor_tensor(out=out[:], in0=out[:], in1=r_extra[:],
                                        op=AOT.max)
            nc.vector.tensor_scalar(out=out[:], in0=out[:],
                                    scalar1=emask_sb[:, cb:cb + 1],
                                    scalar2=None, op0=AOT.mult)
            nc.sync.dma_start(out=do["cfeat"].ap()[cb * 128:(cb + 1) * 128, :],
                              in_=out[:])

    nc.compile()
    return nc


_prog_cache = {}


def _get_prog(cap):
    if cap not in _prog_cache:
        _prog_cache[cap] = build_program(cap)
    return _prog_cache[cap]


def _prep_inputs(feats, cluster_ids, point_ids, centers, offsets, sem, cap):
    counts = np.bincount(cluster_ids.astype(np.int64), minlength=NCLUST).astype(np.int64)
    starts = np.zeros(NCLUST, dtype=np.int64)
    np.cumsum(counts[:-1], out=starts[1:])
    emask = (counts > 0).astype(np.float32)
    k = np.arange(cap)[None, :]
    gidx = starts[:, None] + np.minimum(k, np.maximum(counts, 1)[:, None] - 1)
    pidx = point_ids[gidx].astype(np.int32)

    shifts = (centers + offsets).astype(np.float32)
    semf = sem.astype(np.float32)

    def norms(a):
        f = np.float32
        return ((a[:, 0] * a[:, 0]).astype(f) + (a[:, 1] * a[:, 1]).astype(f)
                ).astype(f) + (a[:, 2] * a[:, 2]).astype(f)

    def cols(arr):
        return np.ascontiguousarray(arr.reshape(NBLK, 128).T)

    in_maps = []
    for d in range(8):
        b, half = d // 2, d % 2
        # permuted point order: my 1024 rows first
        q = np.r_[np.arange(half * HALFP, half * HALFP + HALFP),
                  np.arange((1 - half) * HALFP, (1 - half) * HALFP + HALFP)]
        cv = centers.reshape(B, P, 3)[b][q]
        sv = shifts.reshape(B, P, 3)[b][q]
        smv = semf.reshape(B, P)[b][q]
        m = {"feats": feats}
        cl0 = d * CLPD
        m["pidx"] = np.ascontiguousarray(
            pidx[cl0:cl0 + CLPD].reshape(NCB, 128, cap).transpose(1, 0, 2)
            .reshape(128, NCB * cap))
        m["emask"] = np.ascontiguousarray(emask[cl0:cl0 + CLPD].reshape(NCB, 128).T)
        for pre, view in (("s", sv), ("c", cv)):
            nv = norms(view).astype(np.float32)
            for ax, arr in (("x", view[:, 0]), ("y", view[:, 1]),
                            ("z", view[:, 2]), ("n", nv)):
                arr = np.ascontiguousarray(np.asarray(arr, dtype=np.float32))
                m[f"{pre}{ax}_r"] = arr
                m[f"{pre}{ax}_c"] = cols(arr)
        m["sem_r"] = np.ascontiguousarray(smv)
        m["sem_c"] = cols(smv)
        m["e2_c"] = cols((smv >= 2.0).astype(np.float32))
        for ax in ("x", "y", "z", "n"):
            m[f"dg{ax}"] = np.ascontiguousarray(
                np.concatenate([m[f"s{ax}_c"], m[f"c{ax}_c"]], axis=1))
        in_maps.append(m)
    return in_maps


def kernel(feats, cluster_ids, point_ids, overseg_centers, overseg_offsets,
           overseg_semantic, _trace=False):
    from concourse.bass_utils import run_bass_kernel_spmd

    feats = np.ascontiguousarray(np.asarray(feats, dtype=np.float32))
    cluster_ids = np.asarray(cluster_ids).astype(np.int64)
    point_ids = np.asarray(point_ids).astype(np.int64)
    centers = np.asarray(overseg_centers, dtype=np.float32)
    offsets = np.asarray(overseg_offsets, dtype=np.float32)
    sem = np.asarray(overseg_semantic).astype(np.int64)

    counts = np.bincount(cluster_ids, minlength=NCLUST)
    cap = DEF_CAP if counts.max() <= DEF_CAP else int(32 * np.ceil(counts.max() / 32))

    nc = _get_prog(cap)
    in_maps = _prep_inputs(feats, cluster_ids, point_ids, centers, offsets, sem, cap)
    try:
        res = run_bass_kernel_spmd(nc, in_maps, core_ids=list(range(8)),
                                   trace=_trace)
    except ModuleNotFoundError:
        res = run_bass_kernel_spmd(nc, in_maps, core_ids=list(range(8)),
                                   trace=False)
    outs = res.results

    cluster_feats = np.concatenate([outs[d]["cfeat"] for d in range(8)], axis=0)
    adjacency = np.empty((B, P, P), dtype=np.float32)
    connect = np.empty((B, P, P), dtype=bool)
    for d in range(8):
        b, half = d // 2, d % 2
        r0 = half * HALFP
        adj = outs[d]["adj"]
        conn = outs[d]["conn"].astype(bool)
        if half == 1:   # device columns are in permuted order: un-swap halves
            adj = np.concatenate([adj[:, HALFP:], adj[:, :HALFP]], axis=1)
            conn = np.concatenate([conn[:, HALFP:], conn[:, :HALFP]], axis=1)
        adjacency[b, r0:r0 + HALFP, :] = adj
        connect[b, r0:r0 + HALFP, :] = conn
        dv = outs[d]["diag"][:, :MYBLK]       # my blocks are cols 0..7
        vec = np.ascontiguousarray(dv.T).reshape(HALFP)
        idx = np.arange(r0, r0 + HALFP)
        adjacency[b, idx, idx] = vec
        connect[b, idx, idx] = True
    if _trace:
        kernel._last_results = res
    return cluster_feats, adjacency, connect


# revision 11
# speedup vs baseline: 352764.6501x; 1.4384x over previous
"""PointGroup kernel for 8 trn2 NeuronCores (self-contained).

Reference semantics reproduced bit-exactly:
  1) cluster_feats = segment_max(feats[point_ids], cluster_ids, 4096), with
     empty clusters -> 0.  cluster_ids sorted -> host builds a padded index
     matrix pidx[4096, CAP] (pad = first pid of each cluster so the max is
     unchanged); device gathers rows and tree-max reduces.  Exact.
  2) adjacency/connect per batch (B=4, P=2048).  All decisions are made in
     pre-sqrt (d2) space with rounding sequences matched op-for-op to what
     XLA-CPU emits for the reference (verified bit-exact on host):
       dot_rr : t0=fl(x_i*x_j); t1=fl(fl(y_i*y_j)+t0); t2=fl(fl(z_i*z_j)+t1)
       g      : fl(2*t2 - fl(n_i+n_j))        (negated d2; DVE round-round)
       knn    : g >= 8th-largest in row       (DVE max8 == top_k by value)
       valid  : g >= -A',  A' = largest f32 with fl(sqrt(A')) < 0.3f
       squeeze: g_shift > g_center            (pre-sqrt, validated)
       connect: (sem_i==sem_j) & (sem_i>=2)   (+ eye added on host)
     adjacency = (knn_i|knn_j) & valid & squeeze & connect; the diagonal is
     pure rounding noise (reference uses CPU FMA) and is computed by an
     exact FMA-emulation (Dekker/TwoSum) on per-point columns, output as a
     vector which the host scatters onto the diagonal.
Sharding: device d -> batch d//2, row half d%2 (the device works in a
permuted point order with its 1024 rows first; host un-permutes columns),
plus cluster range [d*512,(d+1)*512) for part 1.
"""
import numpy as np
from contextlib import ExitStack

import concourse.bass as bass
import concourse.bacc as bacc
import concourse.mybir as mybir
from concourse.tile import TileContext

F32 = mybir.dt.float32
U8 = mybir.dt.uint8
I32 = mybir.dt.int32
AOT = mybir.AluOpType
AFT = mybir.ActivationFunctionType

N, C = 262144, 32
NCLUST = 4096
B, P = 4, 2048
NBLK = P // 128          # 16 row blocks per batch
MYBLK = 8                # row blocks owned per device
HALFP = MYBLK * 128      # 1024
CLPD = NCLUST // 8       # clusters per device (512)
NCB = CLPD // 128        # cluster blocks per device (4)
CHUNK = 512              # phase-C column chunk
GSPLIT = 4               # gather sub-passes per cluster block
DEF_CAP = 192


def _a_prime():
    thr = np.float32(0.3)
    x = np.float32(thr * thr)
    while not np.sqrt(x) < thr:
        x = np.nextafter(x, np.float32(0.0), dtype=np.float32)
    while np.sqrt(np.nextafter(x, np.float32(1.0), dtype=np.float32)) < thr:
        x = np.nextafter(x, np.float32(1.0), dtype=np.float32)
    return x


def _emit_fma(nc, wp, y, t, tag):
    """fl(y*y + t) with a single rounding, via exact DVE ops (Dekker/TwoSum).
    y, t: [128, W] SBUF APs.  Returns the result tile."""
    shape = [y.shape[0], y.shape[1]]
    tt, ts = nc.vector.tensor_tensor, nc.vector.tensor_scalar

    def tl(sub):
        return wp.tile(shape, F32, tag=f"fma_{sub}", name=f"fma_{sub}")

    h = tl("h"); tt(out=h[:], in0=y, in1=y, op=AOT.mult)
    c = tl("c"); ts(out=c[:], in0=y, scalar1=4097.0, scalar2=None, op0=AOT.mult)
    d = tl("d"); tt(out=d[:], in0=c[:], in1=y, op=AOT.subtract)
    yh = tl("yh"); tt(out=yh[:], in0=c[:], in1=d[:], op=AOT.subtract)
    yl = tl("yl"); tt(out=yl[:], in0=y, in1=yh[:], op=AOT.subtract)
    p1 = tl("p1"); tt(out=p1[:], in0=yh[:], in1=yh[:], op=AOT.mult)
    e1 = tl("e1"); tt(out=e1[:], in0=p1[:], in1=h[:], op=AOT.subtract)
    p2 = tl("p2"); tt(out=p2[:], in0=yh[:], in1=yl[:], op=AOT.mult)
    p22 = tl("p22"); ts(out=p22[:], in0=p2[:], scalar1=2.0, scalar2=None, op0=AOT.mult)
    e2 = tl("e2"); tt(out=e2[:], in0=e1[:], in1=p22[:], op=AOT.add)
    p3 = tl("p3"); tt(out=p3[:], in0=yl[:], in1=yl[:], op=AOT.mult)
    e = tl("e"); tt(out=e[:], in0=e2[:], in1=p3[:], op=AOT.add)
    s = tl("s"); tt(out=s[:], in0=h[:], in1=t, op=AOT.add)
    z = tl("z"); tt(out=z[:], in0=s[:], in1=h[:], op=AOT.subtract)
    w1 = tl("w1"); tt(out=w1[:], in0=s[:], in1=z[:], op=AOT.subtract)
    w2 = tl("w2"); tt(out=w2[:], in0=h[:], in1=w1[:], op=AOT.subtract)
    w3 = tl("w3"); tt(out=w3[:], in0=t, in1=z[:], op=AOT.subtract)
    er = tl("er"); tt(out=er[:], in0=w2[:], in1=w3[:], op=AOT.add)
    ee = tl("ee"); tt(out=ee[:], in0=er[:], in1=e[:], op=AOT.add)
    r = tl("r"); tt(out=r[:], in0=s[:], in1=ee[:], op=AOT.add)
    return r


def build_program(caps):
    nc = bacc.Bacc("TRN2", target_bir_lowering=False)
    A = float(_a_prime())

    din = {}
    def dram_in(name, shape, dt=F32):
        din[name] = nc.dram_tensor(name, list(shape), dt, kind="ExternalInput")

    dram_in("feats", (N, C))
    CSUM = sum(caps)
    dram_in("pidx", (128, CSUM), I32)
    dram_in("emask", (128, NCB))
    for pre in ("s", "c"):
        for ax in ("x", "y", "z", "n"):
            dram_in(f"{pre}{ax}_r", (P,))
            dram_in(f"{pre}{ax}_c", (128, NBLK))
    dram_in("sem_r", (P,))
    dram_in("sem_c", (128, NBLK))
    dram_in("e2_c", (128, NBLK))
    for ax in ("x", "y", "z", "n"):
        dram_in(f"dg{ax}", (128, 2 * NBLK))

    do = {
        "cfeat": nc.dram_tensor("cfeat", [CLPD, C], F32, kind="ExternalOutput"),
        "adj": nc.dram_tensor("adj", [HALFP, P], F32, kind="ExternalOutput"),
        "conn": nc.dram_tensor("conn", [HALFP, P], U8, kind="ExternalOutput"),
        "diag": nc.dram_tensor("diag", [128, NBLK], F32, kind="ExternalOutput"),
    }

    with TileContext(nc) as tc, ExitStack() as ctx:
        cp = ctx.enter_context(tc.tile_pool(name="cp", bufs=1))   # persistent
        wp = ctx.enter_context(tc.tile_pool(name="wp", bufs=2))   # working
        kp = ctx.enter_context(tc.tile_pool(name="kp", bufs=1))   # gsh keep
        gp = ctx.enter_context(tc.tile_pool(name="gp", bufs=2))   # gather
        dr = ctx.enter_context(tc.tile_pool(name="dr", bufs=1, space="DRAM"))

        def bcast(name, tag):
            t = cp.tile([128, P], F32, tag=tag, name=tag)
            nc.sync.dma_start(out=t[:],
                              in_=din[name].ap()[None, :].to_broadcast([128, P]))
            return t

        def loadt(name, dt=F32, w=NBLK):
            t = cp.tile([128, w], dt, tag=name, name=name)
            nc.sync.dma_start(out=t[:], in_=din[name].ap())
            return t

        # ---------- phase E first: segment max (gpsimd overlaps later DVE) --
        pidx_sb = loadt("pidx", dt=I32, w=CSUM)
        emask_sb = loadt("emask", w=NCB)
        coffs = [0]
        for cpv in caps:
            coffs.append(coffs[-1] + cpv)
        for cb in range(NCB):
            cap_b = caps[cb]
            sub = cap_b // GSPLIT
            parts = []
            for gs in range(GSPLIT):
                G = gp.tile([128, sub * C], F32, tag="G")
                for k in range(sub):
                    kk = coffs[cb] + gs * sub + k
                    nc.gpsimd.indirect_dma_start(
                        out=G[:, k * C:(k + 1) * C], out_offset=None,
                        in_=din["feats"].ap(),
                        in_offset=bass.IndirectOffsetOnAxis(
                            ap=pidx_sb[:, kk:kk + 1], axis=0))
                h = sub
                while h > 1:
                    if h % 2 == 0:
                        h //= 2
                        nc.vector.tensor_tensor(out=G[:, :h * C], in0=G[:, :h * C],
                                                in1=G[:, h * C:2 * h * C], op=AOT.max)
                    else:
                        nc.vector.tensor_tensor(out=G[:, :C], in0=G[:, :C],
                                                in1=G[:, (h - 1) * C:h * C],
                                                op=AOT.max)
                        h -= 1
                r = wp.tile([128, C], F32, tag=f"gr{gs}", name=f"gr{gs}")
                nc.vector.tensor_copy(out=r[:], in_=G[:, :C])
                parts.append(r)
            out = wp.tile([128, C], F32, tag="cfo")
            nc.vector.tensor_tensor(out=out[:], in0=parts[0][:], in1=parts[1][:],
                                    op=AOT.max)
            for r_extra in parts[2:]:
                nc.vector.tensor_tensor(out=out[:], in0=out[:], in1=r_extra[:],
                                        op=AOT.max)
            nc.vector.tensor_scalar(out=out[:], in0=out[:],
                                    scalar1=emask_sb[:, cb:cb + 1],
                                    scalar2=None, op0=AOT.mult)
            nc.sync.dma_start(out=do["cfeat"].ap()[cb * 128:(cb + 1) * 128, :],
                              in_=out[:])

        XJ = bcast("sx_r", "bx"); YJ = bcast("sy_r", "by")
        ZJ = bcast("sz_r", "bz"); NJ = bcast("sn_r", "bn")
        SEMJ = bcast("sem_r", "bsem")
        sx_c = loadt("sx_c"); sy_c = loadt("sy_c")
        sz_c = loadt("sz_c"); sn_c = loadt("sn_c")
        cx_c = loadt("cx_c"); cy_c = loadt("cy_c")
        cz_c = loadt("cz_c"); cn_c = loadt("cn_c")
        sem_c = loadt("sem_c"); e2_c = loadt("e2_c")
        gk8_dram = dr.tile([P], F32)

        def chain(xj, yj, zj, nj, xc, yc, zc, n_c, blk, out_ap, cols=slice(0, P)):
            """out = fl(2*dot_rr - fl(n_i+n_j)) for row-block blk, cols."""
            w = cols.stop - cols.start
            t0 = wp.tile([128, P], F32, tag="t0")
            nc.scalar.activation(out=t0[:, :w], in_=xj[:, cols], func=AFT.Copy,
                                 bias=0.0, scale=xc[:, blk:blk + 1])
            t1 = wp.tile([128, P], F32, tag="t1")
            nc.vector.scalar_tensor_tensor(out=t1[:, :w], in0=yj[:, cols],
                                           scalar=yc[:, blk:blk + 1], in1=t0[:, :w],
                                           op0=AOT.mult, op1=AOT.add)
            t2 = wp.tile([128, P], F32, tag="t0")
            nc.vector.scalar_tensor_tensor(out=t2[:, :w], in0=zj[:, cols],
                                           scalar=zc[:, blk:blk + 1], in1=t1[:, :w],
                                           op0=AOT.mult, op1=AOT.add)
            S = wp.tile([128, P], F32, tag="t1")
            nc.scalar.activation(out=S[:, :w], in_=nj[:, cols], func=AFT.Relu,
                                 bias=n_c[:, blk:blk + 1], scale=1.0)
            nc.vector.scalar_tensor_tensor(out=out_ap, in0=t2[:, :w], scalar=2.0,
                                           in1=S[:, :w], op0=AOT.mult,
                                           op1=AOT.subtract)

        # ---------- phase A: g_shift + kth for all 16 row blocks ----------
        gsh_keep, gk8_cols = [], []
        for blk in range(NBLK):
            keep = blk < MYBLK
            g = (kp.tile([128, P], F32, tag=f"gk{blk}", name=f"gk{blk}") if keep
                 else wp.tile([128, P], F32, tag="gtmp", name="gtmp", bufs=1))
            chain(XJ, YJ, ZJ, NJ, sx_c, sy_c, sz_c, sn_c, blk, g[:])
            m8 = wp.tile([128, 8], F32, tag="m8")
            nc.vector.max(out=m8[:], in_=g[:])
            gcol = (kp.tile([128, 1], F32, tag=f"g8_{blk}", name=f"g8_{blk}") if keep
                    else wp.tile([128, 1], F32, tag="g8tmp", name="g8tmp"))
            nc.vector.tensor_scalar(out=gcol[:], in0=m8[:, 7:8], scalar1=-A,
                                    scalar2=None, op0=AOT.max)
            if keep:
                gsh_keep.append(g); gk8_cols.append(gcol)
            nc.sync.dma_start(out=gk8_dram[blk * 128:(blk + 1) * 128, None],
                              in_=gcol[:])

        GK8 = cp.tile([128, P], F32, tag="bgk8")
        nc.sync.dma_start(out=GK8[:],
                          in_=gk8_dram[None, :].to_broadcast([128, P]))

        # c-side broadcasts reuse the (now dead) shift slots
        XJc = bcast("cx_r", "bx"); YJc = bcast("cy_r", "by")
        ZJc = bcast("cz_r", "bz"); NJc = bcast("cn_r", "bn")

        # ---------- phase C: masks + outputs for my 8 row blocks ----------
        for blk in range(MYBLK):
            gsh, g8c = gsh_keep[blk], gk8_cols[blk]
            for ci in range(P // CHUNK):
                cols = slice(ci * CHUNK, (ci + 1) * CHUNK)
                gc = wp.tile([128, CHUNK], F32, tag="gc")
                chain(XJc, YJc, ZJc, NJc, cx_c, cy_c, cz_c, cn_c, blk,
                      gc[:], cols)
                kc = wp.tile([128, CHUNK], F32, tag="kc")
                nc.vector.tensor_tensor(out=kc[:], in0=GK8[:, cols],
                                        in1=g8c[:, :1].to_broadcast([128, CHUNK]),
                                        op=AOT.min)
                am = wp.tile([128, CHUNK], F32, tag="am")
                nc.vector.tensor_tensor(out=am[:], in0=gsh[:, cols], in1=kc[:],
                                        op=AOT.is_ge)
                cr = wp.tile([128, CHUNK], F32, tag="cr")
                nc.vector.tensor_tensor(out=cr[:], in0=gsh[:, cols], in1=gc[:],
                                        op=AOT.is_gt)
                e12 = wp.tile([128, CHUNK], F32, tag="e12")
                nc.vector.tensor_scalar(out=e12[:], in0=SEMJ[:, cols],
                                        scalar1=sem_c[:, blk:blk + 1],
                                        scalar2=e2_c[:, blk:blk + 1],
                                        op0=AOT.is_equal, op1=AOT.mult)
                ta = wp.tile([128, CHUNK], F32, tag="ta")
                nc.vector.tensor_tensor(out=ta[:], in0=am[:], in1=cr[:], op=AOT.add)
                adjf = wp.tile([128, CHUNK], F32, tag="adjf")
                nc.vector.scalar_tensor_tensor(out=adjf[:], in0=ta[:], scalar=1.5,
                                               in1=e12[:], op0=AOT.is_ge,
                                               op1=AOT.mult)
                nc.sync.dma_start(
                    out=do["adj"].ap()[blk * 128:(blk + 1) * 128, cols], in_=adjf[:])
                cu8 = wp.tile([128, CHUNK], U8, tag="cu8")
                nc.scalar.copy(out=cu8[:], in_=e12[:])
                nc.sync.dma_start(
                    out=do["conn"].ap()[blk * 128:(blk + 1) * 128, cols], in_=cu8[:])

        # ---------- phase D: exact diagonal ----------
        dgx = loadt("dgx", w=2 * NBLK); dgy = loadt("dgy", w=2 * NBLK)
        dgz = loadt("dgz", w=2 * NBLK); dgn = loadt("dgn", w=2 * NBLK)
        t0 = wp.tile([128, 2 * NBLK], F32, tag="fma_t0")
        nc.vector.tensor_tensor(out=t0[:], in0=dgx[:], in1=dgx[:], op=AOT.mult)
        f1 = _emit_fma(nc, wp, dgy[:], t0[:], "dg1")
        f2 = _emit_fma(nc, wp, dgz[:], f1[:], "dg2")
        u1 = wp.tile([128, 2 * NBLK], F32, tag="fma_u1")
        nc.vector.tensor_scalar(out=u1[:], in0=dgn[:], scalar1=2.0,
                                scalar2=None, op0=AOT.mult)
        u2 = wp.tile([128, 2 * NBLK], F32, tag="fma_u2")
        nc.vector.tensor_scalar(out=u2[:], in0=f2[:], scalar1=2.0,
                                scalar2=None, op0=AOT.mult)
        d2 = wp.tile([128, 2 * NBLK], F32, tag="fma_d2")
        nc.vector.tensor_tensor(out=d2[:], in0=u1[:], in1=u2[:], op=AOT.subtract)
        mm = wp.tile([128, 2 * NBLK], F32, tag="fma_m")
        nc.vector.tensor_scalar(out=mm[:], in0=d2[:], scalar1=0.0,
                                scalar2=None, op0=AOT.max)
        dpred = wp.tile([128, NBLK], F32, tag="dpred")
        nc.vector.tensor_tensor(out=dpred[:], in0=mm[:, :NBLK],
                                in1=mm[:, NBLK:], op=AOT.is_lt)
        nc.sync.dma_start(out=do["diag"].ap(), in_=dpred[:])

    nc.compile()
    return nc


_prog_cache = {}


def _get_prog(caps):
    if caps not in _prog_cache:
        _prog_cache[caps] = build_program(caps)
    return _prog_cache[caps]


def _cluster_layout(counts):
    """Global sort by count desc -> shared per-block caps + permutation."""
    perm = np.argsort(-counts, kind="stable")
    caps = []
    for cb in range(NCB):
        blk = perm[cb * 1024:(cb + 1) * 1024]
        cap_b = int(counts[blk].max()) if len(blk) else 1
        caps.append(max(4, 4 * ((cap_b + 3) // 4)))
    return perm, tuple(caps)


def _prep_inputs(feats, cluster_ids, point_ids, centers, offsets, sem,
                 perm, caps):
    counts = np.bincount(cluster_ids.astype(np.int64), minlength=NCLUST).astype(np.int64)
    starts = np.zeros(NCLUST, dtype=np.int64)
    np.cumsum(counts[:-1], out=starts[1:])
    emask = (counts > 0).astype(np.float32)

    shifts = (centers + offsets).astype(np.float32)
    semf = sem.astype(np.float32)

    def norms(a):
        f = np.float32
        return ((a[:, 0] * a[:, 0]).astype(f) + (a[:, 1] * a[:, 1]).astype(f)
                ).astype(f) + (a[:, 2] * a[:, 2]).astype(f)

    def cols(arr):
        return np.ascontiguousarray(arr.reshape(NBLK, 128).T)

    in_maps = []
    for d in range(8):
        b, half = d // 2, d % 2
        # permuted point order: my 1024 rows first
        q = np.r_[np.arange(half * HALFP, half * HALFP + HALFP),
                  np.arange((1 - half) * HALFP, (1 - half) * HALFP + HALFP)]
        cv = centers.reshape(B, P, 3)[b][q]
        sv = shifts.reshape(B, P, 3)[b][q]
        smv = semf.reshape(B, P)[b][q]
        m = {"feats": feats}
        # clusters for (d, cb, p) = perm[cb*1024 + d*128 + p]; per-block caps
        cols_list = []
        em_cols = []
        for cb in range(NCB):
            cl = perm[cb * 1024 + d * 128: cb * 1024 + (d + 1) * 128]
            cap_b = caps[cb]
            k = np.arange(cap_b)[None, :]
            gidx = starts[cl][:, None] + np.minimum(
                k, np.maximum(counts[cl], 1)[:, None] - 1)
            cols_list.append(point_ids[gidx].astype(np.int32))   # [128, cap_b]
            em_cols.append(emask[cl])
        m["pidx"] = np.ascontiguousarray(np.concatenate(cols_list, axis=1))
        m["emask"] = np.ascontiguousarray(np.stack(em_cols, axis=1))
        for pre, view in (("s", sv), ("c", cv)):
            nv = norms(view).astype(np.float32)
            for ax, arr in (("x", view[:, 0]), ("y", view[:, 1]),
                            ("z", view[:, 2]), ("n", nv)):
                arr = np.ascontiguousarray(np.asarray(arr, dtype=np.float32))
                m[f"{pre}{ax}_r"] = arr
                m[f"{pre}{ax}_c"] = cols(arr)
        m["sem_r"] = np.ascontiguousarray(smv)
        m["sem_c"] = cols(smv)
        m["e2_c"] = cols((smv >= 2.0).astype(np.float32))
        for ax in ("x", "y", "z", "n"):
            m[f"dg{ax}"] = np.ascontiguousarray(
                np.concatenate([m[f"s{ax}_c"], m[f"c{ax}_c"]], axis=1))
        in_maps.append(m)
    return in_maps


def kernel(feats, cluster_ids, point_ids, overseg_centers, overseg_offsets,
           overseg_semantic, _trace=False):
    from concourse.bass_utils import run_bass_kernel_spmd

    feats = np.ascontiguousarray(np.asarray(feats, dtype=np.float32))
    cluster_ids = np.asarray(cluster_ids).astype(np.int64)
    point_ids = np.asarray(point_ids).astype(np.int64)
    centers = np.asarray(overseg_centers, dtype=np.float32)
    offsets = np.asarray(overseg_offsets, dtype=np.float32)
    sem = np.asarray(overseg_semantic).astype(np.int64)

    counts = np.bincount(cluster_ids, minlength=NCLUST)
    perm, caps = _cluster_layout(counts)

    nc = _get_prog(caps)
    in_maps = _prep_inputs(feats, cluster_ids, point_ids, centers, offsets, sem,
                           perm, caps)
    try:
        res = run_bass_kernel_spmd(nc, in_maps, core_ids=list(range(8)),
                                   trace=_trace)
    except ModuleNotFoundError:
        res = run_bass_kernel_spmd(nc, in_maps, core_ids=list(range(8)),
                                   trace=False)
    outs = res.results

    cluster_feats = np.empty((NCLUST, C), dtype=np.float32)
    for d in range(8):
        for cb in range(NCB):
            cl = perm[cb * 1024 + d * 128: cb * 1024 + (d + 1) * 128]
            cluster_feats[cl] = outs[d]["cfeat"][cb * 128:(cb + 1) * 128]
    adjacency = np.empty((B, P, P), dtype=np.float32)
    connect = np.empty((B, P, P), dtype=bool)
    for d in range(8):
        b, half = d // 2, d % 2
        r0 = half * HALFP
        adj = outs[d]["adj"]
        conn = outs[d]["conn"].astype(bool)
        if half == 1:   # device columns are in permuted order: un-swap halves
            adj = np.concatenate([adj[:, HALFP:], adj[:, :HALFP]], axis=1)
            conn = np.concatenate([conn[:, HALFP:], conn[:, :HALFP]], axis=1)
        adjacency[b, r0:r0 + HALFP, :] = adj
        connect[b, r0:r0 + HALFP, :] = conn
        dv = outs[d]["diag"][:, :MYBLK]       # my blocks are cols 0..7
        vec = np.ascontiguousarray(dv.T).reshape(HALFP)
        idx = np.arange(r0, r0 + HALFP)
        adjacency[b, idx, idx] = vec
        connect[b, idx, idx] = True
    if _trace:
        kernel._last_results = res
    return cluster_feats, adjacency, connect


# revision 12
# speedup vs baseline: 354804.6403x; 1.0058x over previous
"""PointGroup kernel for 8 trn2 NeuronCores (self-contained).

Reference semantics reproduced bit-exactly:
  1) cluster_feats = segment_max(feats[point_ids], cluster_ids, 4096), with
     empty clusters -> 0.  cluster_ids sorted -> host builds a padded index
     matrix pidx[4096, CAP] (pad = first pid of each cluster so the max is
     unchanged); device gathers rows and tree-max reduces.  Exact.
  2) adjacency/connect per batch (B=4, P=2048).  All decisions are made in
     pre-sqrt (d2) space with rounding sequences matched op-for-op to what
     XLA-CPU emits for the reference (verified bit-exact on host):
       dot_rr : t0=fl(x_i*x_j); t1=fl(fl(y_i*y_j)+t0); t2=fl(fl(z_i*z_j)+t1)
       g      : fl(2*t2 - fl(n_i+n_j))        (negated d2; DVE round-round)
       knn    : g >= 8th-largest in row       (DVE max8 == top_k by value)
       valid  : g >= -A',  A' = largest f32 with fl(sqrt(A')) < 0.3f
       squeeze: g_shift > g_center            (pre-sqrt, validated)
       connect: (sem_i==sem_j) & (sem_i>=2)   (+ eye added on host)
     adjacency = (knn_i|knn_j) & valid & squeeze & connect; the diagonal is
     pure rounding noise (reference uses CPU FMA) and is computed by an
     exact FMA-emulation (Dekker/TwoSum) on per-point columns, output as a
     vector which the host scatters onto the diagonal.
Sharding: device d -> batch d//2, row half d%2 (the device works in a
permuted point order with its 1024 rows first; host un-permutes columns),
plus cluster range [d*512,(d+1)*512) for part 1.
"""
import numpy as np
from contextlib import ExitStack

import concourse.bass as bass
import concourse.bacc as bacc
import concourse.mybir as mybir
from concourse.tile import TileContext

F32 = mybir.dt.float32
U8 = mybir.dt.uint8
I32 = mybir.dt.int32
AOT = mybir.AluOpType
AFT = mybir.ActivationFunctionType

N, C = 262144, 32
NCLUST = 4096
B, P = 4, 2048
NBLK = P // 128          # 16 row blocks per batch
MYBLK = 8                # row blocks owned per device
HALFP = MYBLK * 128      # 1024
CLPD = NCLUST // 8       # clusters per device (512)
NCB = CLPD // 128        # cluster blocks per device (4)
CHUNK = 512              # phase-C column chunk
GSPLIT = 4               # gather sub-passes per cluster block
DEF_CAP = 192


def _a_prime():
    thr = np.float32(0.3)
    x = np.float32(thr * thr)
    while not np.sqrt(x) < thr:
        x = np.nextafter(x, np.float32(0.0), dtype=np.float32)
    while np.sqrt(np.nextafter(x, np.float32(1.0), dtype=np.float32)) < thr:
        x = np.nextafter(x, np.float32(1.0), dtype=np.float32)
    return x


def _emit_fma(nc, wp, y, t, tag):
    """fl(y*y + t) with a single rounding, via exact DVE ops (Dekker/TwoSum).
    y, t: [128, W] SBUF APs.  Returns the result tile."""
    shape = [y.shape[0], y.shape[1]]
    tt, ts = nc.vector.tensor_tensor, nc.vector.tensor_scalar

    def tl(sub):
        return wp.tile(shape, F32, tag=f"fma_{sub}", name=f"fma_{sub}")

    h = tl("h"); tt(out=h[:], in0=y, in1=y, op=AOT.mult)
    c = tl("c"); ts(out=c[:], in0=y, scalar1=4097.0, scalar2=None, op0=AOT.mult)
    d = tl("d"); tt(out=d[:], in0=c[:], in1=y, op=AOT.subtract)
    yh = tl("yh"); tt(out=yh[:], in0=c[:], in1=d[:], op=AOT.subtract)
    yl = tl("yl"); tt(out=yl[:], in0=y, in1=yh[:], op=AOT.subtract)
    p1 = tl("p1"); tt(out=p1[:], in0=yh[:], in1=yh[:], op=AOT.mult)
    e1 = tl("e1"); tt(out=e1[:], in0=p1[:], in1=h[:], op=AOT.subtract)
    p2 = tl("p2"); tt(out=p2[:], in0=yh[:], in1=yl[:], op=AOT.mult)
    p22 = tl("p22"); ts(out=p22[:], in0=p2[:], scalar1=2.0, scalar2=None, op0=AOT.mult)
    e2 = tl("e2"); tt(out=e2[:], in0=e1[:], in1=p22[:], op=AOT.add)
    p3 = tl("p3"); tt(out=p3[:], in0=yl[:], in1=yl[:], op=AOT.mult)
    e = tl("e"); tt(out=e[:], in0=e2[:], in1=p3[:], op=AOT.add)
    s = tl("s"); tt(out=s[:], in0=h[:], in1=t, op=AOT.add)
    z = tl("z"); tt(out=z[:], in0=s[:], in1=h[:], op=AOT.subtract)
    w1 = tl("w1"); tt(out=w1[:], in0=s[:], in1=z[:], op=AOT.subtract)
    w2 = tl("w2"); tt(out=w2[:], in0=h[:], in1=w1[:], op=AOT.subtract)
    w3 = tl("w3"); tt(out=w3[:], in0=t, in1=z[:], op=AOT.subtract)
    er = tl("er"); tt(out=er[:], in0=w2[:], in1=w3[:], op=AOT.add)
    ee = tl("ee"); tt(out=ee[:], in0=er[:], in1=e[:], op=AOT.add)
    r = tl("r"); tt(out=r[:], in0=s[:], in1=ee[:], op=AOT.add)
    return r


def build_program(caps):
    nc = bacc.Bacc("TRN2", target_bir_lowering=False)
    A = float(_a_prime())

    din = {}
    def dram_in(name, shape, dt=F32):
        din[name] = nc.dram_tensor(name, list(shape), dt, kind="ExternalInput")

    dram_in("feats", (N, C))
    CSUM = sum(caps)
    dram_in("pidx", (128, CSUM), I32)
    dram_in("emask", (128, NCB))
    for pre in ("s", "c"):
        for ax in ("x", "y", "z", "n"):
            dram_in(f"{pre}{ax}_r", (P,))
            dram_in(f"{pre}{ax}_c", (128, NBLK))
    dram_in("sem_r", (P,))
    dram_in("sem_c", (128, NBLK))
    dram_in("e2_c", (128, NBLK))
    for ax in ("x", "y", "z", "n"):
        dram_in(f"dg{ax}", (128, 2 * NBLK))

    do = {
        "cfeat": nc.dram_tensor("cfeat", [CLPD, C], F32, kind="ExternalOutput"),
        "adj": nc.dram_tensor("adj", [HALFP, P], F32, kind="ExternalOutput"),
        "conn": nc.dram_tensor("conn", [HALFP, P], U8, kind="ExternalOutput"),
        "diag": nc.dram_tensor("diag", [128, NBLK], F32, kind="ExternalOutput"),
    }

    with TileContext(nc) as tc, ExitStack() as ctx:
        cp = ctx.enter_context(tc.tile_pool(name="cp", bufs=1))   # persistent
        wp = ctx.enter_context(tc.tile_pool(name="wp", bufs=2))   # working
        kp = ctx.enter_context(tc.tile_pool(name="kp", bufs=1))   # gsh keep
        gp = ctx.enter_context(tc.tile_pool(name="gp", bufs=2))   # gather
        dr = ctx.enter_context(tc.tile_pool(name="dr", bufs=1, space="DRAM"))

        def bcast(name, tag):
            t = cp.tile([128, P], F32, tag=tag, name=tag)
            nc.sync.dma_start(out=t[:],
                              in_=din[name].ap()[None, :].to_broadcast([128, P]))
            return t

        def loadt(name, dt=F32, w=NBLK):
            t = cp.tile([128, w], dt, tag=name, name=name)
            nc.sync.dma_start(out=t[:], in_=din[name].ap())
            return t

        # ---------- phase E first: segment max (gpsimd overlaps later DVE) --
        pidx_sb = loadt("pidx", dt=I32, w=CSUM)
        emask_sb = loadt("emask", w=NCB)
        coffs = [0]
        for cpv in caps:
            coffs.append(coffs[-1] + cpv)
        def emit_gather_block(cb):
            cap_b = caps[cb]
            sub = cap_b // GSPLIT
            parts = []
            for gs in range(GSPLIT):
                G = gp.tile([128, sub * C], F32, tag="G")
                for k in range(sub):
                    kk = coffs[cb] + gs * sub + k
                    nc.gpsimd.indirect_dma_start(
                        out=G[:, k * C:(k + 1) * C], out_offset=None,
                        in_=din["feats"].ap(),
                        in_offset=bass.IndirectOffsetOnAxis(
                            ap=pidx_sb[:, kk:kk + 1], axis=0))
                h = sub
                while h > 1:
                    if h % 2 == 0:
                        h //= 2
                        nc.vector.tensor_tensor(out=G[:, :h * C], in0=G[:, :h * C],
                                                in1=G[:, h * C:2 * h * C], op=AOT.max)
                    else:
                        nc.vector.tensor_tensor(out=G[:, :C], in0=G[:, :C],
                                                in1=G[:, (h - 1) * C:h * C],
                                                op=AOT.max)
                        h -= 1
                r = wp.tile([128, C], F32, tag=f"gr{gs}", name=f"gr{gs}")
                nc.vector.tensor_copy(out=r[:], in_=G[:, :C])
                parts.append(r)
            out = wp.tile([128, C], F32, tag="cfo")
            nc.vector.tensor_tensor(out=out[:], in0=parts[0][:], in1=parts[1][:],
                                    op=AOT.max)
            for r_extra in parts[2:]:
                nc.vector.tensor_tensor(out=out[:], in0=out[:], in1=r_extra[:],
                                        op=AOT.max)
            nc.vector.tensor_scalar(out=out[:], in0=out[:],
                                    scalar1=emask_sb[:, cb:cb + 1],
                                    scalar2=None, op0=AOT.mult)
            nc.sync.dma_start(out=do["cfeat"].ap()[cb * 128:(cb + 1) * 128, :],
                              in_=out[:])


        XJ = bcast("sx_r", "bx"); YJ = bcast("sy_r", "by")
        ZJ = bcast("sz_r", "bz"); NJ = bcast("sn_r", "bn")
        SEMJ = bcast("sem_r", "bsem")
        sx_c = loadt("sx_c"); sy_c = loadt("sy_c")
        sz_c = loadt("sz_c"); sn_c = loadt("sn_c")
        cx_c = loadt("cx_c"); cy_c = loadt("cy_c")
        cz_c = loadt("cz_c"); cn_c = loadt("cn_c")
        sem_c = loadt("sem_c"); e2_c = loadt("e2_c")
        gk8_dram = dr.tile([P], F32)

        def chain(xj, yj, zj, nj, xc, yc, zc, n_c, blk, out_ap, cols=slice(0, P)):
            """out = fl(2*dot_rr - fl(n_i+n_j)) for row-block blk, cols."""
            w = cols.stop - cols.start
            t0 = wp.tile([128, P], F32, tag="t0")
            nc.scalar.activation(out=t0[:, :w], in_=xj[:, cols], func=AFT.Copy,
                                 bias=0.0, scale=xc[:, blk:blk + 1])
            t1 = wp.tile([128, P], F32, tag="t1")
            nc.vector.scalar_tensor_tensor(out=t1[:, :w], in0=yj[:, cols],
                                           scalar=yc[:, blk:blk + 1], in1=t0[:, :w],
                                           op0=AOT.mult, op1=AOT.add)
            t2 = wp.tile([128, P], F32, tag="t0")
            nc.vector.scalar_tensor_tensor(out=t2[:, :w], in0=zj[:, cols],
                                           scalar=zc[:, blk:blk + 1], in1=t1[:, :w],
                                           op0=AOT.mult, op1=AOT.add)
            S = wp.tile([128, P], F32, tag="t1")
            nc.scalar.activation(out=S[:, :w], in_=nj[:, cols], func=AFT.Relu,
                                 bias=n_c[:, blk:blk + 1], scale=1.0)
            nc.vector.scalar_tensor_tensor(out=out_ap, in0=t2[:, :w], scalar=2.0,
                                           in1=S[:, :w], op0=AOT.mult,
                                           op1=AOT.subtract)

        # ---------- phase A: g_shift + kth for all 16 row blocks ----------
        gsh_keep, gk8_cols = [], []
        for blk in range(NBLK):
            if blk % 4 == 0:
                emit_gather_block(blk // 4)
            keep = blk < MYBLK
            g = (kp.tile([128, P], F32, tag=f"gk{blk}", name=f"gk{blk}") if keep
                 else wp.tile([128, P], F32, tag="gtmp", name="gtmp", bufs=1))
            chain(XJ, YJ, ZJ, NJ, sx_c, sy_c, sz_c, sn_c, blk, g[:])
            m8 = wp.tile([128, 8], F32, tag="m8")
            nc.vector.max(out=m8[:], in_=g[:])
            gcol = (kp.tile([128, 1], F32, tag=f"g8_{blk}", name=f"g8_{blk}") if keep
                    else wp.tile([128, 1], F32, tag="g8tmp", name="g8tmp"))
            nc.vector.tensor_scalar(out=gcol[:], in0=m8[:, 7:8], scalar1=-A,
                                    scalar2=None, op0=AOT.max)
            if keep:
                gsh_keep.append(g); gk8_cols.append(gcol)
            nc.sync.dma_start(out=gk8_dram[blk * 128:(blk + 1) * 128, None],
                              in_=gcol[:])

        GK8 = cp.tile([128, P], F32, tag="bgk8")
        nc.sync.dma_start(out=GK8[:],
                          in_=gk8_dram[None, :].to_broadcast([128, P]))

        # c-side broadcasts reuse the (now dead) shift slots
        XJc = bcast("cx_r", "bx"); YJc = bcast("cy_r", "by")
        ZJc = bcast("cz_r", "bz"); NJc = bcast("cn_r", "bn")

        # ---------- phase C: masks + outputs for my 8 row blocks ----------
        for blk in range(MYBLK):
            gsh, g8c = gsh_keep[blk], gk8_cols[blk]
            for ci in range(P // CHUNK):
                cols = slice(ci * CHUNK, (ci + 1) * CHUNK)
                gc = wp.tile([128, CHUNK], F32, tag="gc")
                chain(XJc, YJc, ZJc, NJc, cx_c, cy_c, cz_c, cn_c, blk,
                      gc[:], cols)
                kc = wp.tile([128, CHUNK], F32, tag="kc")
                nc.vector.tensor_tensor(out=kc[:], in0=GK8[:, cols],
                                        in1=g8c[:, :1].to_broadcast([128, CHUNK]),
                                        op=AOT.min)
                am = wp.tile([128, CHUNK], F32, tag="am")
                nc.vector.tensor_tensor(out=am[:], in0=gsh[:, cols], in1=kc[:],
                                        op=AOT.is_ge)
                cr = wp.tile([128, CHUNK], F32, tag="cr")
                nc.vector.tensor_tensor(out=cr[:], in0=gsh[:, cols], in1=gc[:],
                                        op=AOT.is_gt)
                e12 = wp.tile([128, CHUNK], F32, tag="e12")
                nc.vector.tensor_scalar(out=e12[:], in0=SEMJ[:, cols],
                                        scalar1=sem_c[:, blk:blk + 1],
                                        scalar2=e2_c[:, blk:blk + 1],
                                        op0=AOT.is_equal, op1=AOT.mult)
                ta = wp.tile([128, CHUNK], F32, tag="ta")
                nc.vector.tensor_tensor(out=ta[:], in0=am[:], in1=cr[:], op=AOT.add)
                adjf = wp.tile([128, CHUNK], F32, tag="adjf")
                nc.vector.scalar_tensor_tensor(out=adjf[:], in0=ta[:], scalar=1.5,
                                               in1=e12[:], op0=AOT.is_ge,
                                               op1=AOT.mult)
                nc.sync.dma_start(
                    out=do["adj"].ap()[blk * 128:(blk + 1) * 128, cols], in_=adjf[:])
                cu8 = wp.tile([128, CHUNK], U8, tag="cu8")
                nc.scalar.copy(out=cu8[:], in_=e12[:])
                nc.sync.dma_start(
                    out=do["conn"].ap()[blk * 128:(blk + 1) * 128, cols], in_=cu8[:])

        # ---------- phase D: exact diagonal ----------
        dgx = loadt("dgx", w=2 * NBLK); dgy = loadt("dgy", w=2 * NBLK)
        dgz = loadt("dgz", w=2 * NBLK); dgn = loadt("dgn", w=2 * NBLK)
        t0 = wp.tile([128, 2 * NBLK], F32, tag="fma_t0")
        nc.vector.tensor_tensor(out=t0[:], in0=dgx[:], in1=dgx[:], op=AOT.mult)
        f1 = _emit_fma(nc, wp, dgy[:], t0[:], "dg1")
        f2 = _emit_fma(nc, wp, dgz[:], f1[:], "dg2")
        u1 = wp.tile([128, 2 * NBLK], F32, tag="fma_u1")
        nc.vector.tensor_scalar(out=u1[:], in0=dgn[:], scalar1=2.0,
                                scalar2=None, op0=AOT.mult)
        u2 = wp.tile([128, 2 * NBLK], F32, tag="fma_u2")
        nc.vector.tensor_scalar(out=u2[:], in0=f2[:], scalar1=2.0,
                                scalar2=None, op0=AOT.mult)
        d2 = wp.tile([128, 2 * NBLK], F32, tag="fma_d2")
        nc.vector.tensor_tensor(out=d2[:], in0=u1[:], in1=u2[:], op=AOT.subtract)
        mm = wp.tile([128, 2 * NBLK], F32, tag="fma_m")
        nc.vector.tensor_scalar(out=mm[:], in0=d2[:], scalar1=0.0,
                                scalar2=None, op0=AOT.max)
        dpred = wp.tile([128, NBLK], F32, tag="dpred")
        nc.vector.tensor_tensor(out=dpred[:], in0=mm[:, :NBLK],
                                in1=mm[:, NBLK:], op=AOT.is_lt)
        nc.sync.dma_start(out=do["diag"].ap(), in_=dpred[:])

    nc.compile()
    return nc


_prog_cache = {}


def _get_prog(caps):
    if caps not in _prog_cache:
        _prog_cache[caps] = build_program(caps)
    return _prog_cache[caps]


def _cluster_layout(counts):
    """Global sort by count desc -> shared per-block caps + permutation."""
    perm = np.argsort(-counts, kind="stable")
    caps = []
    for cb in range(NCB):
        blk = perm[cb * 1024:(cb + 1) * 1024]
        cap_b = int(counts[blk].max()) if len(blk) else 1
        caps.append(max(4, 4 * ((cap_b + 3) // 4)))
    return perm, tuple(caps)


def _prep_inputs(feats, cluster_ids, point_ids, centers, offsets, sem,
                 perm, caps):
    counts = np.bincount(cluster_ids.astype(np.int64), minlength=NCLUST).astype(np.int64)
    starts = np.zeros(NCLUST, dtype=np.int64)
    np.cumsum(counts[:-1], out=starts[1:])
    emask = (counts > 0).astype(np.float32)

    shifts = (centers + offsets).astype(np.float32)
    semf = sem.astype(np.float32)

    def norms(a):
        f = np.float32
        return ((a[:, 0] * a[:, 0]).astype(f) + (a[:, 1] * a[:, 1]).astype(f)
                ).astype(f) + (a[:, 2] * a[:, 2]).astype(f)

    def cols(arr):
        return np.ascontiguousarray(arr.reshape(NBLK, 128).T)

    in_maps = []
    for d in range(8):
        b, half = d // 2, d % 2
        # permuted point order: my 1024 rows first
        q = np.r_[np.arange(half * HALFP, half * HALFP + HALFP),
                  np.arange((1 - half) * HALFP, (1 - half) * HALFP + HALFP)]
        cv = centers.reshape(B, P, 3)[b][q]
        sv = shifts.reshape(B, P, 3)[b][q]
        smv = semf.reshape(B, P)[b][q]
        m = {"feats": feats}
        # clusters for (d, cb, p) = perm[cb*1024 + d*128 + p]; per-block caps
        cols_list = []
        em_cols = []
        for cb in range(NCB):
            cl = perm[cb * 1024 + d * 128: cb * 1024 + (d + 1) * 128]
            cap_b = caps[cb]
            k = np.arange(cap_b)[None, :]
            gidx = starts[cl][:, None] + np.minimum(
                k, np.maximum(counts[cl], 1)[:, None] - 1)
            cols_list.append(point_ids[gidx].astype(np.int32))   # [128, cap_b]
            em_cols.append(emask[cl])
        m["pidx"] = np.ascontiguousarray(np.concatenate(cols_list, axis=1))
        m["emask"] = np.ascontiguousarray(np.stack(em_cols, axis=1))
        for pre, view in (("s", sv), ("c", cv)):
            nv = norms(view).astype(np.float32)
            for ax, arr in (("x", view[:, 0]), ("y", view[:, 1]),
                            ("z", view[:, 2]), ("n", nv)):
                arr = np.ascontiguousarray(np.asarray(arr, dtype=np.float32))
                m[f"{pre}{ax}_r"] = arr
                m[f"{pre}{ax}_c"] = cols(arr)
        m["sem_r"] = np.ascontiguousarray(smv)
        m["sem_c"] = cols(smv)
        m["e2_c"] = cols((smv >= 2.0).astype(np.float32))
        for ax in ("x", "y", "z", "n"):
            m[f"dg{ax}"] = np.ascontiguousarray(
                np.concatenate([m[f"s{ax}_c"], m[f"c{ax}_c"]], axis=1))
        in_maps.append(m)
    return in_maps


def kernel(feats, cluster_ids, point_ids, overseg_centers, overseg_offsets,
           overseg_semantic, _trace=False):
    from concourse.bass_utils import run_bass_kernel_spmd

    feats = np.ascontiguousarray(np.asarray(feats, dtype=np.float32))
    cluster_ids = np.asarray(cluster_ids).astype(np.int64)
    point_ids = np.asarray(point_ids).astype(np.int64)
    centers = np.asarray(overseg_centers, dtype=np.float32)
    offsets = np.asarray(overseg_offsets, dtype=np.float32)
    sem = np.asarray(overseg_semantic).astype(np.int64)

    counts = np.bincount(cluster_ids, minlength=NCLUST)
    perm, caps = _cluster_layout(counts)

    nc = _get_prog(caps)
    in_maps = _prep_inputs(feats, cluster_ids, point_ids, centers, offsets, sem,
                           perm, caps)
    try:
        res = run_bass_kernel_spmd(nc, in_maps, core_ids=list(range(8)),
                                   trace=_trace)
    except ModuleNotFoundError:
        res = run_bass_kernel_spmd(nc, in_maps, core_ids=list(range(8)),
                                   trace=False)
    outs = res.results

    cluster_feats = np.empty((NCLUST, C), dtype=np.float32)
    for d in range(8):
        for cb in range(NCB):
            cl = perm[cb * 1024 + d * 128: cb * 1024 + (d + 1) * 128]
            cluster_feats[cl] = outs[d]["cfeat"][cb * 128:(cb + 1) * 128]
    adjacency = np.empty((B, P, P), dtype=np.float32)
    connect = np.empty((B, P, P), dtype=bool)
    for d in range(8):
        b, half = d // 2, d % 2
        r0 = half * HALFP
        adj = outs[d]["adj"]
        conn = outs[d]["conn"].astype(bool)
        if half == 1:   # device columns are in permuted order: un-swap halves
            adj = np.concatenate([adj[:, HALFP:], adj[:, :HALFP]], axis=1)
            conn = np.concatenate([conn[:, HALFP:], conn[:, :HALFP]], axis=1)
        adjacency[b, r0:r0 + HALFP, :] = adj
        connect[b, r0:r0 + HALFP, :] = conn
        dv = outs[d]["diag"][:, :MYBLK]       # my blocks are cols 0..7
        vec = np.ascontiguousarray(dv.T).reshape(HALFP)
        idx = np.arange(r0, r0 + HALFP)
        adjacency[b, idx, idx] = vec
        connect[b, idx, idx] = True
    if _trace:
        kernel._last_results = res
    return cluster_feats, adjacency, connect
